# revision 59
# baseline (speedup 1.0000x reference)
"""GQA causal-attention prefill kernel for 8 Trainium2 NeuronCores.

Reference computation (B=2, S=2048, D=4096, Q=32 q-heads, N=8 kv-heads,
H=128): QKV projection + RoPE + causal GQA attention + O projection.

Sharding: core c handles batch b = c//4 and kv-head pair g = c%4
(kv-heads 2g..2g+1, q-heads 8g..8g+7).  No collectives: each core
computes its partial o-projection (sum over its 8 q-heads) and the host
sums the four partials per batch at gather time (the "all-reduce").

v4 design (v2 + DMA-layout / queue / ring-depth / pipeline work;
measured 728-731us vs the 809us v2 baseline, PE ~96% busy):
  - all weights and x are pre-arranged on the host into the exact
    per-partition SBUF layout, so every weight load is one contiguous
    DMA (8-16KB descriptor lines instead of 256B) and x chunks are
    contiguous 16KB-per-partition runs.
  - startup is HBM-bandwidth-bound (~220 GB/s effective with all 8
    cores pulling): the whole j=0 working set is issued on the sync
    queue alone, in exact need order (x chunk0, wk0, wk1, cos, sin,
    x chunk1, wq0, wv, wq1); tiny tables ride the scalar queue; a
    run of dummy matmuls on the perm table warms the PE HAM clock
    gate while the first DMAs land.
  - steady-state queues: sync=x chunks + wo e-chunks, gpsimd=wq +
    o_out writes, scalar=one-time tables.
  - softmax tiles for diagonal (partially-masked) t-tiles live in 3
    persistent SBUF buffers whose below-diagonal columns are zeroed
    once and never written again -> no per-tile DVE memsets.
  - attention kt-loop is software-pipelined: AV/den/o-proj-sprinkle
    trail scores+exp by one t-tile so the AV matmul never waits on
    the scalar engine's exp.
  - the o-proj output ring (oc tiles) has 5 buffers: the PSUM->SBUF
    copy WARs on the o_out DMA of 5 tiles back (~5-7us SWDGE
    completion latency), which with 2 buffers serialized PE via the
    pc PSUM ring (this was the single biggest stall, ~40us).
  - AV matmuls on diagonal tiles only touch their valid s >= t
    columns (accumulating psum writes at column offset lo work fine).
  - denominators: per-kt ones-matmuls at j=0 (keeps PE above the exp
    rate with no filler), 8-tile DVE sum trees + one ones-matmul per
    group for j>0; 1/den via fast DVE reciprocal applied at the
    normalize (PSUM->SBUF) multiply.
  - v-projection sits between q-head 0 and q-head 1 so wv's 2MB DMA
    is off the j=0 critical path; v/psum copies run on ScalarE, o-proj
    psum copies on DVE at high scheduler priority.
  - x chunk pool has 3 buffers (next j's first chunk prefetches during
    the current j) and the attention-output buffer is a 2-slice ring
    indexed by j parity (o-proj reads parity j-1 while attention
    writes parity j), trading 16KB of ot for the extra x buffer.
  - bf16 data path end-to-end (cos/sin/tri tables bf16 too), fp32
    PSUM accumulation, scores computed transposed (S^T = K^T Q) so
    softmax weights feed AV directly; RoPE rotate-half as a TensorE
    matmul with a constant permutation matrix (sign folded into sin).
  - o-projection for s-tile j-1 is sprinkled one matmul per t-tile
    into s-tile j's attention to keep PE above the exp rate.
"""

import math
import sys

import numpy as np

for _p in ("/opt/trn_rl_repo", "/root/.axon_site/_ro/trn_rl_repo"):
    if _p not in sys.path:
        sys.path.append(_p)

import ml_dtypes

import concourse.bacc as bacc
import concourse.mybir as mybir
import concourse.tile as tile
from concourse import bass_utils

dt = mybir.dt
F32 = dt.float32
BF16 = dt.bfloat16
ADD = mybir.AluOpType.add
MULT = mybir.AluOpType.mult
EXP = mybir.ActivationFunctionType.Exp
COPY = mybir.ActivationFunctionType.Copy
NP_BF16 = ml_dtypes.bfloat16

# Full-problem config (per core after sharding).
FULL_CFG = dict(S=2048, D=4096, QH=8, KH=2, H=128, SC=256, ST=512)
N_CORES = 8
ROPE_THETA = 10000.0
NEG_BIG = -1.0e30


def build_bass(cfg):
    S, D, QH, KH, H = cfg["S"], cfg["D"], cfg["QH"], cfg["KH"], cfg["H"]
    SC, ST = cfg["SC"], cfg["ST"]
    assert H == 128 and D % 128 == 0 and S % SC == 0 and SC % 128 == 0
    assert S % ST == 0 and ST % 128 == 0 and QH % KH == 0
    DT = D // 128          # d-tiles (contraction tiles for projections)
    NJ = S // ST           # s-tiles for attention / main loop
    TJ = ST // 128         # 128-wide t-tiles per attention s-tile
    NT = S // 128          # total t-tiles
    CPJ = ST // SC         # x-chunks per s-tile
    NCH = S // SC          # total x-chunks
    G = QH // KH           # GQA group size
    EW = 512               # o-proj output tile width
    NE = D // EW
    scale = 1.0 / math.sqrt(H)

    nc = bacc.Bacc("TRN2", target_bir_lowering=False, debug=False,
                   enable_asserts=False, num_devices=N_CORES)

    # host pre-arranged layouts (partition dim always first/exact)
    xc = nc.dram_tensor("xc", [128, NCH, DT, SC], BF16, kind="ExternalInput")
    wq = nc.dram_tensor("wq", [QH, 128, DT, H], BF16, kind="ExternalInput")
    wk = nc.dram_tensor("wk", [KH, 128, DT, H], BF16, kind="ExternalInput")
    wv = nc.dram_tensor("wv", [128, DT, KH, H], BF16, kind="ExternalInput")
    wo = nc.dram_tensor("wo", [128, NE, QH, EW], BF16, kind="ExternalInput")
    cos_d = nc.dram_tensor("cos_t", [128, S], BF16, kind="ExternalInput")
    sin_d = nc.dram_tensor("sin_t", [128, S], BF16, kind="ExternalInput")
    tri_d = nc.dram_tensor("tri_t", [128, 128], BF16, kind="ExternalInput")
    ones_d = nc.dram_tensor("ones_t", [128, 128], BF16, kind="ExternalInput")
    perm_d = nc.dram_tensor("perm_t", [128, 128], BF16, kind="ExternalInput")
    o_out = nc.dram_tensor("o_out", [S, D], BF16, kind="ExternalOutput")

    with tile.TileContext(nc) as tc, \
         nc.allow_low_precision(reason="deliberate bf16 matmul pipeline"):
        with tc.tile_pool(name="persist", bufs=1) as persist:
            cos_sb = persist.tile([128, S], BF16)
            sin_sb = persist.tile([128, S], BF16)
            tri_sb = persist.tile([128, 128], BF16)
            ones_sb = persist.tile([128, 128], BF16)
            perm_sb = persist.tile([128, 128], BF16)
            k_sb = persist.tile([128, KH, S], BF16)
            v_sb = persist.tile([128, NT, KH * H], BF16)
            # attention output ring: 2 j-slices (o-proj for j-1 reads one
            # parity while attention j writes the other)
            ot_sb = persist.tile([128, QH, 2 * ST], BF16)
            wk_t = persist.tile([128, KH, DT, H], BF16)
            wv_t = persist.tile([128, DT, KH, H], BF16)
            # persistent diagonal softmax tiles: below-diagonal columns
            # zeroed once, never written again (exp only writes lo:ST)
            dwt = [persist.tile([128, ST], BF16, name="dwt_%d" % m)
                   for m in range(TJ - 1)]

            def load_tables():
                # one-time loads on the scalar queue (sync carries the x
                # chunks + wv, gpsimd carries wq); wk head 0 + cos/sin
                # first (j=0 k-proj + k-rope critical path)
                for m in range(TJ - 1):
                    nc.vector.memset(dwt[m][:, 0:(m + 1) * 128], 0.0)
                # tiny tables on the otherwise-idle scalar queue; all the
                # j=0 bulk goes on the sync queue in exact need order (a
                # single queue still uses all 16 SDMA engines, and a
                # strict order beats three queues splitting the bandwidth)
                nc.scalar.dma_start(perm_sb[:], perm_d[:, :])
                nc.scalar.dma_start(tri_sb[:], tri_d[:, :])
                nc.scalar.dma_start(ones_sb[:], ones_d[:, :])

            with tc.tile_pool(name="xts", bufs=3) as xtsp, \
                 tc.tile_pool(name="wqp", bufs=3) as wqp, \
                 tc.tile_pool(name="qj", bufs=8) as qjp, \
                 tc.tile_pool(name="wt", bufs=6) as wtp, \
                 tc.tile_pool(name="rope", bufs=2) as rp, \
                 tc.tile_pool(name="qd", bufs=1) as qdp, \
                 tc.tile_pool(name="wop", bufs=4) as wop, \
                 tc.tile_pool(name="ocp", bufs=6) as ocp, \
                 tc.tile_pool(name="rcp", bufs=1) as rcpp, \
                 tc.tile_pool(name="prj", bufs=2, space="PSUM") as prjp, \
                 tc.tile_pool(name="pss", bufs=3, space="PSUM") as pss, \
                 tc.tile_pool(name="pso", bufs=2, space="PSUM") as pso, \
                 tc.tile_pool(name="psd", bufs=1, space="PSUM") as psd:

                def rope(ps_tile, dst_ap, s0, W):
                    """dst = rope(ps_tile) for s-range [s0, s0+W).

                    The rotate-half partition swap runs on TensorE as a
                    matmul with a constant permutation matrix (sign
                    folded into the sin table)."""
                    ta = rp.tile([128, W], BF16, tag="ta")
                    tb = rp.tile([128, W], BF16, tag="tb")
                    csl = cos_sb[:, s0:s0 + W]
                    ssl = sin_sb[:, s0:s0 + W]
                    nc.vector.tensor_tensor(tb[:], ps_tile, ssl, MULT)
                    nc.vector.tensor_tensor(ta[:], ps_tile, csl, MULT)
                    tbs = pss.tile([128, W], F32, tag="ps")
                    nc.tensor.matmul(tbs[:], perm_sb[:], tb[:],
                                     start=True, stop=True)
                    nc.vector.tensor_tensor(dst_ap, ta[:], tbs[:], ADD)

                def attn_head(h, qjt, j, filler):
                    kh = h // G
                    po = pso.tile([128, ST], F32, tag="po")
                    # all-ones stationary -> every psum partition gets the
                    # t-sum, so no denominator broadcast is ever needed.
                    pden = psd.tile([128, ST], F32, tag="pden")
                    KT = (j + 1) * TJ
                    # denominator groups: sum 8 exp-tiles per ones-matmul
                    # where possible (4 for the KT=12 tail) to halve the
                    # PE-side den matmul count
                    dgroups = [8] * (KT // 8) + ([4] if KT % 8 else [])
                    dstate = {"g": 0, "n": 0}
                    wts = []

                    def consume(kt, wtile):
                        # AV / denominator / o-proj sprinkle for tile kt;
                        # runs one kt behind scores+exp so the AV matmul
                        # never waits on the scalar engine's exp.
                        # Diagonal tiles only touch their valid s >= t
                        # columns (cols below lo already hold the full sum
                        # of earlier t-tiles; kt==0 writes all columns).
                        lo = max(kt - j * TJ, 0) * 128
                        vsl = v_sb[:, kt, kh * H:(kh + 1) * H]
                        nc.tensor.matmul(
                            po[:, lo:ST], vsl, wtile[:, lo:ST],
                            start=(kt == 0), stop=(kt == KT - 1))
                        wts.append(wtile)
                        gi = dstate["g"]
                        if len(wts) == dgroups[gi]:
                            qa = qdp.tile([128, ST], BF16, tag="qa")
                            qb = qdp.tile([128, ST], BF16, tag="qb")
                            qc = qdp.tile([128, ST], BF16, tag="qc")
                            nc.vector.tensor_tensor(qa[:], wts[0][:],
                                                    wts[1][:], ADD)
                            nc.vector.tensor_tensor(qb[:], wts[2][:],
                                                    wts[3][:], ADD)
                            nc.vector.tensor_tensor(qc[:], qa[:], qb[:], ADD)
                            mv = qc
                            if len(wts) == 8:
                                qa = qdp.tile([128, ST], BF16, tag="qa")
                                qb = qdp.tile([128, ST], BF16, tag="qb")
                                qd = qdp.tile([128, ST], BF16, tag="qd")
                                nc.vector.tensor_tensor(qa[:], wts[4][:],
                                                        wts[5][:], ADD)
                                nc.vector.tensor_tensor(qb[:], wts[6][:],
                                                        wts[7][:], ADD)
                                nc.vector.tensor_tensor(qd[:], qa[:], qb[:],
                                                        ADD)
                                q8 = qdp.tile([128, ST], BF16, tag="qa")
                                nc.vector.tensor_tensor(q8[:], qc[:], qd[:],
                                                        ADD)
                                mv = q8
                            nc.tensor.matmul(
                                pden[:], ones_sb[:], mv[:],
                                start=(gi == 0), stop=(gi == len(dgroups) - 1))
                            wts.clear()
                            dstate["g"] = gi + 1
                        if filler is not None:
                            next(filler, None)

                    pending = []
                    for kt in range(KT):
                        m = kt - j * TJ
                        lo = max(m, 0) * 128
                        ps = pss.tile([128, ST], F32, tag="ps")
                        # diagonal tiles compute only the valid s >= t
                        # columns, written at column 0 of a fresh psum
                        # tile (hw matmul outputs want offset 0)
                        nc.tensor.matmul(
                            ps[:, 0:ST - lo],
                            k_sb[:, kh, kt * 128:(kt + 1) * 128],
                            qjt[:, lo:ST], start=True, stop=True)
                        if m >= 0:
                            nc.vector.tensor_tensor(
                                ps[:, 0:128], ps[:, 0:128], tri_sb[:], ADD)
                        if m >= 1:
                            wtile = dwt[m - 1]
                        else:
                            wtile = wtp.tile([128, ST], BF16, tag="wt")
                        nc.scalar.activation(wtile[:, lo:ST], ps[:, 0:ST - lo],
                                             EXP, scale=scale)
                        pending.append((kt, wtile))
                        if len(pending) > 1:
                            consume(*pending.pop(0))
                    while pending:
                        consume(*pending.pop(0))
                    rcb = rcpp.tile([128, ST], F32, tag="rcp")
                    nc.vector.reciprocal_approx_fast(rcb[:], pden[:])
                    ob = (j % 2) * ST
                    nc.vector.tensor_tensor(
                        ot_sb[:, h, ob:ob + ST], po[:], rcb[:],
                        MULT)

                class Oproj:
                    # o-projection emitter for s-tile j: wo e-chunks are
                    # single contiguous DMAs alternating scalar/sync
                    # queues; matmuls are yielded one per advance for
                    # sprinkling into the next tile's attention
                    def __init__(self, j):
                        self.j = j
                        self.woes = [None] * NE

                    def load(self, e):
                        t = wop.tile([128, QH, EW], BF16, tag="woe",
                                     name="woe_%d_%d" % (self.j, e))
                        nc.sync.dma_start(t[:], wo.ap()[:, e])
                        self.woes[e] = t

                    def prefetch(self):
                        self.load(0)
                        self.load(1)

                    def gen(self):
                        j = self.j
                        ob = (j % 2) * ST
                        for e in range(NE):
                            for stl in range(TJ):
                                st = j * TJ + stl
                                pc = prjp.tile([128, EW], F32, tag="prj",
                                               name="pc_%d_%d_%d" % (j, e, stl))
                                for hh in range(QH):
                                    nc.tensor.matmul(
                                        pc[:],
                                        ot_sb[:, hh, ob + stl * 128:
                                              ob + (stl + 1) * 128],
                                        self.woes[e][:, hh, :],
                                        start=(hh == 0), stop=(hh == QH - 1))
                                    yield
                                oc = ocp.tile([128, EW], BF16, tag="oc")
                                # high priority: the pc psum ring (WAR) is
                                # released by this copy; schedule it ahead
                                # of unrelated queued vector work
                                with tc.high_priority():
                                    nc.vector.tensor_copy(oc[:], pc[:])
                                eng = (nc.sync if (j == NJ - 1 and e == NE - 1)
                                       else nc.gpsimd)
                                eng.dma_start(
                                    o_out[st * 128:(st + 1) * 128,
                                          e * EW:(e + 1) * EW],
                                    oc[:])
                            if e + 2 < NE:
                                self.load(e + 2)

                load_tables()
                # HAM warm-up: dummy matmuls on the first-loaded tiny
                # table while the j=0 x/wk DMAs stream in, so the real
                # projections start at 2.4 GHz instead of 1.2
                wps = pss.tile([128, 128], F32, tag="ps", name="warm")
                for _ in range(52):
                    nc.tensor.matmul(wps[:], perm_sb[:], perm_sb[:],
                                     start=True, stop=True)
                oproj_prev = None
                for j in range(NJ):
                    if j > 0:
                        oproj_prev = Oproj(j - 1)
                    qj = [qjp.tile([128, ST], BF16, tag="qj",
                                   name="qj%d_%d" % (j, h))
                          for h in range(QH)]
                    xts_tiles = []
                    hdt = DT // 2
                    j0_wq = {}
                    if j == 0:
                        # startup is HBM-bandwidth-bound: issue the whole
                        # j=0 working set on ONE queue in exact need order
                        for half in range(CPJ):
                            xts_tiles.append(
                                xtsp.tile([128, DT, SC], BF16, tag="xts",
                                          name="xts0_%d" % half))
                        nc.sync.dma_start(xts_tiles[0][:, 0:hdt],
                                          xc.ap()[:, 0, 0:hdt])
                        nc.sync.dma_start(wk_t[:, 0], wk.ap()[0])
                        nc.sync.dma_start(xts_tiles[0][:, hdt:DT],
                                          xc.ap()[:, 0, hdt:DT])
                        nc.sync.dma_start(wk_t[:, 1], wk.ap()[1])
                        # sin before cos: the rope's PE perm-matmul needs
                        # the sin product; the cos product only feeds the
                        # final DVE add
                        nc.sync.dma_start(sin_sb[:], sin_d[:, :])
                        nc.sync.dma_start(cos_sb[:], cos_d[:, :])
                        nc.sync.dma_start(xts_tiles[1][:, 0:hdt],
                                          xc.ap()[:, 1, 0:hdt])
                        nc.sync.dma_start(xts_tiles[1][:, hdt:DT],
                                          xc.ap()[:, 1, hdt:DT])
                        for h in range(2):
                            t = wqp.tile([128, DT, H], BF16, tag="wq",
                                         name="wq_0_%d" % h)
                            nc.sync.dma_start(t[:], wq.ap()[h])
                            j0_wq[h] = t
                            if h == 0:
                                nc.sync.dma_start(wv_t[:], wv.ap()[:, :])
                    else:
                        for half in range(CPJ):
                            ch = j * CPJ + half
                            xts = xtsp.tile([128, DT, SC], BF16, tag="xts")
                            nc.sync.dma_start(xts[:, 0:hdt],
                                              xc.ap()[:, ch, 0:hdt])
                            nc.sync.dma_start(xts[:, hdt:DT],
                                              xc.ap()[:, ch, hdt:DT])
                            xts_tiles.append(xts)
                            if half == 0:
                                oproj_prev.prefetch()
                    # k projection per chunk
                    for half in range(CPJ):
                        s0 = (j * CPJ + half) * SC
                        for kh in range(KH):
                            pk = prjp.tile([128, SC], F32, tag="prj")
                            for di in range(DT):
                                nc.tensor.matmul(
                                    pk[:], wk_t[:, kh, di, :],
                                    xts_tiles[half][:, di, :],
                                    start=(di == 0), stop=(di == DT - 1))
                            rope(pk[:], k_sb[:, kh, s0:s0 + SC], s0, SC)

                    # q projection head-outer (wq ring releases slots in
                    # load order); v projection after head 0 so the wv DMA
                    # sits after wq0 in the j=0 need order
                    def qproj(h):
                        if h in j0_wq:
                            wt_ = j0_wq.pop(h)
                        else:
                            wt_ = wqp.tile([128, DT, H], BF16, tag="wq",
                                           name="wq_%d_%d" % (j, h))
                            eng = nc.sync if j == 0 else nc.gpsimd
                            eng.dma_start(wt_[:], wq.ap()[h])
                        for half in range(CPJ):
                            s0 = (j * CPJ + half) * SC
                            pq = prjp.tile([128, SC], F32, tag="prj")
                            for di in range(DT):
                                nc.tensor.matmul(
                                    pq[:], wt_[:, di, :],
                                    xts_tiles[half][:, di, :],
                                    start=(di == 0), stop=(di == DT - 1))
                            rope(pq[:], qj[h][:, half * SC:(half + 1) * SC],
                                 s0, SC)

                    qproj(0)
                    for half in range(CPJ):
                        ch = j * CPJ + half
                        for tl in range(SC // 128):
                            pv = prjp.tile([128, KH * H], F32, tag="prj")
                            for di in range(DT):
                                nc.tensor.matmul(
                                    pv[:],
                                    xts_tiles[half][:, di,
                                                    tl * 128:(tl + 1) * 128],
                                    wv_t[:, di].rearrange("p a b -> p (a b)"),
                                    start=(di == 0), stop=(di == DT - 1))
                            tt = ch * (SC // 128) + tl
                            nc.scalar.activation(v_sb[:, tt, :], pv[:], COPY)
                    for h in range(1, QH):
                        qproj(h)
                    filler = oproj_prev.gen() if j > 0 else None
                    for h in range(QH):
                        attn_head(h, qj[h], j, filler)
                    if filler is not None:
                        for _ in filler:
                            pass
                # final s-tile o-projection (dense tail)
                op_last = Oproj(NJ - 1)
                op_last.prefetch()
                for _ in op_last.gen():
                    pass

    nc.compile()
    return nc


def _perm_matrix():
    P = np.zeros((128, 128), dtype=np.float32)
    P[np.arange(128), (np.arange(128) + 64) % 128] = 1.0
    return P


def make_tables(positions_b, S, H):
    """cos/sin tables in [128, S] layout with the sign fold for the swap
    trick (rows 0:63 -> +sin, 64:127 -> -sin), plus the triangular mask."""
    half = H // 2
    inv_freq = 1.0 / (ROPE_THETA ** (np.arange(half, dtype=np.float64) * 2.0 / H))
    ang = positions_b.astype(np.float64)[None, :] * inv_freq[:, None]  # [half, S]
    cos_h = np.cos(ang)
    sin_h = np.sin(ang)
    cos_t = np.concatenate([cos_h, cos_h], axis=0).astype(NP_BF16)
    sin_t = np.concatenate([sin_h, -sin_h], axis=0).astype(NP_BF16)
    idx = np.arange(128)
    tri = np.where(idx[:, None] <= idx[None, :], 0.0, NEG_BIG).astype(NP_BF16)
    return cos_t, sin_t, tri


def make_in_maps(x, positions, Wq, Wk, Wv, Wo, cfg):
    """Shard the full inputs into the 8 per-core input maps (bf16),
    pre-arranged into the exact SBUF layouts the kernel DMAs from."""
    QH, KH = cfg["QH"], cfg["KH"]
    S, H, D, SC = cfg["S"], cfg["H"], cfg["D"], cfg["SC"]
    DT = D // 128
    NCH = S // SC
    EW = 512
    NE = D // EW
    B = x.shape[0]
    groups = N_CORES // B
    tables = [make_tables(np.asarray(positions[b]), S, H) for b in range(B)]
    # x chunk-major: [128(p), NCH, DT, SC]; xc[p, ch, dt, s] = x[ch*SC+s, dt*128+p]
    xc_b = []
    for b in range(B):
        xb = np.asarray(x[b]).astype(NP_BF16)            # [S, D]
        xr = xb.reshape(NCH, SC, DT, 128).transpose(3, 0, 2, 1)
        xc_b.append(np.ascontiguousarray(xr))
    def warr(W):  # [D, H] -> [128, DT, H]
        return np.ascontiguousarray(
            W.reshape(DT, 128, H).transpose(1, 0, 2).astype(NP_BF16))
    wq_g = [np.stack([warr(Wq[g * QH + h]) for h in range(QH)])
            for g in range(groups)]
    wk_g = [np.stack([warr(Wk[g * KH + n]) for n in range(KH)])
            for g in range(groups)]
    # wv: [128, DT, KH, H]
    wv_g = [np.ascontiguousarray(
                np.stack([warr(Wv[g * KH + n]) for n in range(KH)], axis=2))
            for g in range(groups)]
    # wo: [128(H), NE, QH, EW]; Wo[h] is [H, D]
    wo_g = [np.ascontiguousarray(
                np.stack([Wo[g * QH + h].reshape(H, NE, EW).astype(NP_BF16)
                          for h in range(QH)], axis=2))
            for g in range(groups)]
    in_maps = []
    for c in range(N_CORES):
        b, g = divmod(c, groups)
        cos_t, sin_t, tri = tables[b]
        in_maps.append({
            "xc": xc_b[b],
            "wq": wq_g[g],
            "wk": wk_g[g],
            "wv": wv_g[g],
            "wo": wo_g[g],
            "cos_t": cos_t,
            "sin_t": sin_t,
            "tri_t": tri,
            "ones_t": np.ones((128, 128), dtype=NP_BF16),
            "perm_t": _perm_matrix().astype(NP_BF16),
        })
    return in_maps


_NC_CACHE = {}


def _get_nc(cfg_key=None):
    cfg = FULL_CFG if cfg_key is None else cfg_key
    key = tuple(sorted(cfg.items()))
    if key not in _NC_CACHE:
        _NC_CACHE[key] = build_bass(cfg)
    return _NC_CACHE[key]


def run(x, positions, Wq, Wk, Wv, Wo, trace=False, trace_kwargs=None):
    cfg = FULL_CFG
    nc = _get_nc(cfg)
    in_maps = make_in_maps(np.asarray(x), np.asarray(positions),
                           np.asarray(Wq), np.asarray(Wk), np.asarray(Wv),
                           np.asarray(Wo), cfg)
    res = bass_utils.run_bass_kernel_spmd(
        nc, in_maps, list(range(N_CORES)), trace=trace,
        **(trace_kwargs or {}))
    B = np.asarray(x).shape[0]
    groups = N_CORES // B
    outs = []
    for b in range(B):
        acc = res.results[b * groups]["o_out"].astype(np.float64)
        for g in range(1, groups):
            acc += res.results[b * groups + g]["o_out"].astype(np.float64)
        outs.append(acc.astype(np.float32))
    return np.stack(outs, axis=0), res


def kernel(x, positions, Wq, Wk, Wv, Wo):
    out, _ = run(x, positions, Wq, Wk, Wv, Wo, trace=False)
    return out
